# revision 1
# baseline (speedup 1.0000x reference)
"""Trainium2 Bass kernel for nn_CrossAttention_47502338294587.

Math: the reference cross-attention has a single KV position broadcast over
all T query positions.  Softmax over a row of identical logits is uniform,
so attention output == v for every query, and the whole module collapses to

    out[b, t, :] = (visual_features[b] @ Wv + bv) @ Wp + bp      (for all t)

independent of x / Wq / Wk.  The device computes the two projections and
broadcasts the per-batch row over the T axis; the host only does input
layout prep and shard re-assembly (pure data movement, no arithmetic).

Sharding: tensor-parallel over the output channel dim C — core i computes
and writes out[:, :, i*128:(i+1)*128] (it loads full Wv but only its column
shard of Wp / bp).  With C-sharding, a core's whole output shard is one
[128, B*128] tile replicated over the 8 t-chunks, so the T-broadcast is a
single selector matmul + one replicated DMA.

Per-core structure:
  mm1:   vv = vf @ Wv          stationary vf^T chunks, moving Wv (N=512)
         + bv fused into the PSUM->SBUF copy (DVE tensor_add)
  tr:    vv^T chunks via PE transpose
  mm2:   row_sh = vv @ Wp[:,ci] (+ bp[ci] fused into copy)
  bcast: rhs4[k, b*128+c] = row_sh[k,c]*(k==b)  (DVE), then
         bc[t, (b,c)] = ones^T @ rhs4 (one matmul),
         one DMA with a step-0 replicated source writes all 8 t-chunks
"""

import os
import sys

import numpy as np

for _p in ("/opt/trn_rl_repo",):
    if _p not in sys.path and os.path.isdir(_p):
        sys.path.insert(0, _p)

B, T, C = 4, 1024, 1024
N_CORES = 8
CSH = C // N_CORES  # 128, C-shard per core
KC = C // 128  # 8 contraction chunks

_BUILT = None


def build_nc():
    """Build + compile the Bass program (one NeuronCore's SPMD body)."""
    import concourse.bass as bass
    import concourse.mybir as mybir
    import concourse.tile as tile
    from concourse import bacc
    from concourse.bass import ts

    f32 = mybir.dt.float32
    nc = bacc.Bacc("TRN2", target_bir_lowering=False, debug=False)

    wv = nc.dram_tensor("wv", [C, C], f32, kind="ExternalInput")
    # host pre-packs these into the exact SBUF layouts (pure layout prep):
    wp_p = nc.dram_tensor("wp_p", [128, KC * CSH], f32, kind="ExternalInput")
    vft_p = nc.dram_tensor("vft_p", [128, KC * B], f32, kind="ExternalInput")
    bv4 = nc.dram_tensor("bv4", [B, C], f32, kind="ExternalInput")
    bp4sel = nc.dram_tensor("bp4sel", [B, B * CSH], f32, kind="ExternalInput")
    # out[t, b, c_local]; host re-assembles full[b, t, ci] = out[t, b, :]
    out = nc.dram_tensor("out", [T, B, CSH], f32, kind="ExternalOutput")

    def band_select(ap, mult, width):
        """keep 1.0 inside the band 0 <= y - mult*k <= width-1, else 0."""
        nc.gpsimd.memset(ap, 1.0)
        nc.gpsimd.affine_select(
            out=ap, in_=ap, compare_op=mybir.AluOpType.is_ge, fill=0.0,
            base=0, pattern=[[1, ap.shape[-1]]], channel_multiplier=-mult,
        )
        nc.gpsimd.affine_select(
            out=ap, in_=ap, compare_op=mybir.AluOpType.is_ge, fill=0.0,
            base=width - 1, pattern=[[-1, ap.shape[-1]]], channel_multiplier=mult,
        )

    with tile.TileContext(nc) as tc:
        with tc.tile_pool(name="sb", bufs=1) as sb:
            # ---- SBUF tiles -------------------------------------------------
            wv_t = [sb.tile([128, C], f32, name=f"wv{k}", tag=f"wv{k}") for k in range(KC)]
            wp_t = sb.tile([128, KC, CSH], f32, tag="wp_t")
            vft_t = sb.tile([128, KC, B], f32, tag="vft")
            bv4_t = sb.tile([B, C], f32, tag="bv4")
            bp4sel_t = sb.tile([B, B * CSH], f32, tag="bp4sel")
            ones_bp = sb.tile([B, 128], f32, tag="ones_bp")
            # sel[k, b*128 + c] = (k == b)
            sel_t = sb.tile([B, B * 128], f32, tag="sel")
            ident_t = sb.tile([B, B], f32, tag="ident")
            vv_sb = sb.tile([B, C], f32, tag="vv_sb")
            vvt_t = sb.tile([128, KC * B], f32, tag="vvt")
            rhs4_t = sb.tile([B, B * CSH], f32, tag="rhs4")
            bc_t = sb.tile([128, B * CSH], f32, tag="bc")

            nc.vector.memset(ones_bp[:], 1.0)
            band_select(sel_t[:], 128, 128)
            band_select(ident_t[:], 1, 1)

            # ---- DMA in (first mm1 dependency first) ------------------------
            nc.scalar.dma_start(vft_t[:], vft_p.rearrange("p (k b) -> p k b", b=B))
            nc.scalar.dma_start(bv4_t[:], bv4[:, :])
            nc.scalar.dma_start(bp4sel_t[:], bp4sel[:, :])
            nc.scalar.dma_start(wp_t[:], wp_p.rearrange("p (k c) -> p k c", c=CSH))
            nc.sync.dma_start(wv_t[0][:, 0:512], wv[ts(0, 128), 0:512])
            nc.sync.dma_start(wv_t[0][:, 512:1024], wv[ts(0, 128), 512:1024])
            for k in range(1, KC):
                nc.sync.dma_start(wv_t[k][:], wv[ts(k, 128), :])

            # ---- mm1: vv[b, n] = sum_k vf[b, k] Wv[k, n]  (+bv via DVE) -----
            with tc.tile_pool(name="pv", bufs=2, space="PSUM") as pv:
                psum_vv = [pv.tile([B, 512], f32, name=f"pvv{h}", tag=f"pvv{h}") for h in range(2)]
                for k in range(KC):
                    for h in range(2):
                        nc.tensor.matmul(
                            psum_vv[h][:],
                            vft_t[:, k, :],
                            wv_t[k][:, ts(h, 512)],
                            start=(k == 0),
                            stop=(k == KC - 1),
                        )
                for h in range(2):
                    nc.vector.tensor_add(
                        vv_sb[0:B, ts(h, 512)], psum_vv[h][:], bv4_t[0:B, ts(h, 512)]
                    )

            # ---- transpose vv -> vv^T chunks [128, B] -----------------------
            with tc.tile_pool(name="pt", bufs=4, space="PSUM") as pt:
                for k in range(KC):
                    psum_vvt = pt.tile([128, B], f32, tag="pvt")
                    nc.tensor.transpose(
                        psum_vvt[:], vv_sb[0:B, ts(k, 128)], ident_t[0:B, 0:B]
                    )
                    nc.vector.tensor_copy(vvt_t[:, ts(k, B)], psum_vvt[:])

            # ---- mm2: row_sh = vv @ Wp[:,ci]  (+bp via DVE) -----------------
            with (
                tc.tile_pool(name="pr", bufs=1, space="PSUM") as pr,
                tc.tile_pool(name="pb", bufs=1, space="PSUM") as pb,
            ):
                psum_row = pr.tile([B, CSH], f32, tag="pr")
                for k in range(KC):
                    nc.tensor.matmul(
                        psum_row[:],
                        vvt_t[:, ts(k, B)],
                        wp_t[:, k, :],
                        start=(k == 0),
                        stop=(k == KC - 1),
                    )
                # ---- broadcast: one [128, B*CSH] tile == whole shard --------
                # rhs4 = rep4(psum_row) * sel + bp4sel   (bp fused via host-packed
                # block-diagonal bp4sel; rep4 = step-0 replicated AP)
                pra = psum_row[:]
                prep = bass.AP(
                    pra.tensor, pra.offset, [list(pra.ap[0]), [0, B], list(pra.ap[1])]
                )
                nc.vector.tensor_mul(
                    rhs4_t[:].rearrange("p (q f) -> p q f", q=B),
                    prep,
                    sel_t[:].rearrange("p (q f) -> p q f", q=B),
                )
                nc.vector.tensor_add(rhs4_t[:], rhs4_t[:], bp4sel_t[:])
                psum_bc = pb.tile([128, B * CSH], f32, tag="pb")
                for i in range(2):
                    nc.tensor.matmul(
                        psum_bc[:, i * 256 : (i + 1) * 256],
                        ones_bp[0:B, :],
                        rhs4_t[0:B, i * 256 : (i + 1) * 256],
                        start=True,
                        stop=True,
                    )
                # split copy + replicated out-DMA into b-halves on separate
                # queues so the first half's write starts earlier
                half = B * CSH // 2
                out_v = out.rearrange("(q p) b c -> p q (b c)", p=128)
                for i, eng in ((0, nc.sync), (1, nc.scalar)):
                    nc.vector.tensor_copy(
                        bc_t[:, i * half : (i + 1) * half],
                        psum_bc[:, i * half : (i + 1) * half],
                    )
                    ap = bc_t[:, i * half : (i + 1) * half]
                    rep = bass.AP(
                        ap.tensor, ap.offset, [list(ap.ap[0]), [0, KC], list(ap.ap[1])]
                    )
                    eng.dma_start(out_v[:, :, i * half : (i + 1) * half], rep)

    nc.compile()
    return nc


def _get_built():
    global _BUILT
    if _BUILT is None:
        _BUILT = build_nc()
    return _BUILT


def make_in_maps(inputs):
    vf = np.asarray(inputs["visual_features"], np.float32)
    wv = np.ascontiguousarray(np.asarray(inputs["Wv"], np.float32))
    wp = np.asarray(inputs["Wp"], np.float32)
    bv = np.asarray(inputs["bv"], np.float32)
    bp = np.asarray(inputs["bp"], np.float32)
    # vft_p[p, k*B + b] = vf[b, k*128 + p]
    vft_p = np.ascontiguousarray(
        vf.T.reshape(KC, 128, B).transpose(1, 0, 2).reshape(128, KC * B)
    )
    bv4 = np.ascontiguousarray(np.broadcast_to(bv[None, :], (B, C)))
    maps = []
    for i in range(N_CORES):
        ci = slice(i * CSH, (i + 1) * CSH)
        # wp_p[p, k*CSH + c] = Wp[k*128 + p, ci_c]
        wp_p = np.ascontiguousarray(
            wp[:, ci].reshape(KC, 128, CSH).transpose(1, 0, 2).reshape(128, KC * CSH)
        )
        bp4sel = np.zeros((B, B * CSH), np.float32)
        for b in range(B):
            bp4sel[b, b * CSH : (b + 1) * CSH] = bp[ci]
        maps.append(
            {"wv": wv, "wp_p": wp_p, "vft_p": vft_p, "bv4": bv4, "bp4sel": bp4sel}
        )
    return maps


def run(inputs, trace=False, **kw):
    from concourse.bass_utils import run_bass_kernel_spmd

    nc = _get_built()
    res = run_bass_kernel_spmd(
        nc,
        make_in_maps(inputs),
        core_ids=list(range(N_CORES)),
        trace=trace,
        **kw,
    )
    full = np.empty((B, T, C), np.float32)
    for i, r in enumerate(res.results):
        full[:, :, i * CSH : (i + 1) * CSH] = r["out"].transpose(1, 0, 2)
    return full, res


def kernel(**inputs) -> np.ndarray:
    full, _ = run(inputs, trace=False)
    return full



# revision 4
# speedup vs baseline: 1.3200x; 1.3200x over previous
"""Trainium2 Bass kernel for nn_CrossAttention_47502338294587.

Math: the reference cross-attention has a single KV position broadcast over
all T query positions.  Softmax over a row of identical logits is uniform,
so attention output == v for every query, and the whole module collapses to

    out[b, t, :] = (visual_features[b] @ Wv + bv) @ Wp + bp      (for all t)

independent of x / Wq / Wk.  The device computes the two projections and
broadcasts the per-batch row over the T axis; the host only does input
layout prep (incl. bf16 weight packing) and shard re-assembly.

Sharding: tensor-parallel over the output channel dim C - core i computes
and writes out[:, :, i*128:(i+1)*128] (full Wv, column shard of Wp / bp).

v2 vs v1: weights in bf16 (half the DMA bytes, single-pass matmuls instead
of fp32 LOW_HIGH two-pass), mm1 chunks pipelined behind the two HWDGE DMA
queues, constants host-packed (no gpsimd memsets / affine_selects), bv
folded into the mm1 PSUM accumulation via a K=1 matmul, bp folded into the
broadcast matmul via a 5th contraction row, output DMA with 2KB descriptors
split by t-chunk halves across both queues.

Per-core structure:
  mm1:   psum_vv[h] = bv (K=1 matmul) + sum_k vfT_k^T @ Wv_k[:, h]   (bf16)
  tr:    vv -> vv^T chunks via PE transpose (bf16)
  mm2:   prow = sum_k vvT_k^T @ Wp_k          [4, 128] f32 psum
  bcast: rhs5[0:4] = rep4(prow) * sel (DVE), rhs5[4] = bp row (host const)
         pbc[t, (b,c)] = ones5^T @ rhs5  (one fp32 matmul, K=5)
         two replicated-source DMAs (t-chunks 0-3 / 4-7) write the shard
"""

import os
import sys

import numpy as np

for _p in ("/opt/trn_rl_repo",):
    if _p not in sys.path and os.path.isdir(_p):
        sys.path.insert(0, _p)

B, T, C = 4, 1024, 1024
N_CORES = 8
CSH = C // N_CORES  # 128, C-shard per core
KC = C // 128  # 8 contraction chunks

_BUILT = None


def build_nc():
    """Build + compile the Bass program (one NeuronCore's SPMD body)."""
    import concourse.bass as bass
    import concourse.mybir as mybir
    import concourse.tile as tile
    from concourse import bacc
    from concourse.bass import ts

    f32 = mybir.dt.float32
    bf16 = mybir.dt.bfloat16
    nc = bacc.Bacc("TRN2", target_bir_lowering=False, debug=False)

    # ---- DRAM inputs (host pre-packed layouts) --------------------------
    # wv_k[p, n] = bf16(Wv[k*128 + p, n])
    wv_d = [
        nc.dram_tensor(f"wv{k}", [128, C], bf16, kind="ExternalInput")
        for k in range(KC)
    ]
    # vfti[p, 0:32]   = vfT chunks: [p, k*4 + b] = vf[b, k*128 + p]
    # vfti[0:4, 32:36] = eye(4)
    # vfti[0:1, 36:1060] = bv row
    # vfti[0:1, 1060:1064] = ones (lhsT for the K=1 bias matmul)
    vfti_d = nc.dram_tensor("vfti", [128, 1064], bf16, kind="ExternalInput")
    # wp_p[p, k*CSH + c] = bf16(Wp[k*128 + p, ci_c])
    wp_d = nc.dram_tensor("wp_p", [128, KC * CSH], bf16, kind="ExternalInput")
    # consts5 rows 0-3 cols 0:512 = sel ( (k==b) block mask ),
    # row 4 cols 0:512 unused, cols 512:640 all-ones (ones5)
    consts_d = nc.dram_tensor("consts5", [5, 640], f32, kind="ExternalInput")
    # bp_row[(b,c)] = bp[ci_c]  (tiled 4x) -> row 4 of rhs5
    bprow_d = nc.dram_tensor("bp_row", [1, B * CSH], f32, kind="ExternalInput")
    # out[t, b, c_local]; host re-assembles full[b, t, ci] = out[t, b, :]
    out = nc.dram_tensor("out", [T, B, CSH], f32, kind="ExternalOutput")

    with tile.TileContext(nc) as tc:
        with tc.tile_pool(name="sb", bufs=1) as sb:
            # ---- SBUF tiles -------------------------------------------------
            wv_t = [
                sb.tile([128, C], bf16, name=f"wv{k}", tag=f"wv{k}")
                for k in range(KC)
            ]
            vfti_t = sb.tile([128, 1064], bf16, tag="vfti")
            wp_t = sb.tile([128, KC, CSH], bf16, tag="wp_t")
            consts_t = sb.tile([5, 640], f32, tag="consts5")
            # vv halves (bf16), split so transposes 0-3 / 4-7 gate separately
            vv_sb = [sb.tile([B, 512], bf16, name=f"vv{h}", tag=f"vv{h}") for h in range(2)]
            vvt_sb = [sb.tile([128, 4, B], bf16, name=f"vvt{h}", tag=f"vvt{h}") for h in range(2)]
            rhs5_t = sb.tile([5, B * CSH], f32, tag="rhs5")
            bc_t = sb.tile([128, B * CSH], f32, tag="bc")

            vft = vfti_t[:, 0:32].rearrange("p (k b) -> p k b", b=B)
            ident = vfti_t[0:4, 32:36]
            bv_row = vfti_t[0:1, 36:1060]
            ones1 = vfti_t[0:1, 1060:1064]
            sel = consts_t[0:4, 0:512]
            ones5 = consts_t[0:5, 512:640]

            # ---- DMA in: two HWDGE queues, wv chunks alternating ------------
            # sync queue: consts5, bp_row, wv 0/2/4/6 (+ out q0-3 at the end)
            # scalar queue: vfti, wp, wv 1/3/5/7 (+ out q4-7 at the end)
            nc.sync.dma_start(consts_t[:], consts_d[:, :])
            nc.sync.dma_start(rhs5_t[4:5, :], bprow_d[:, :])
            nc.scalar.dma_start(vfti_t[:], vfti_d[:, :])
            nc.scalar.dma_start(
                wp_t[:], wp_d.rearrange("p (k c) -> p k c", c=CSH)
            )
            for k in range(KC):
                eng = nc.sync if k % 2 == 0 else nc.scalar
                eng.dma_start(wv_t[k][:], wv_d[k][:, :])

            # ---- mm1: psum_vv[h] = bv + sum_k vfT_k^T @ Wv_k[:, h] ----------
            with tc.tile_pool(name="pv", bufs=1, space="PSUM") as pv:
                psum_vv = [
                    pv.tile([B, 512], f32, name=f"pvv{h}", tag=f"pvv{h}")
                    for h in range(2)
                ]
                # bias row via K=1 matmul (runs as soon as vfti lands)
                for h in range(2):
                    nc.tensor.matmul(
                        psum_vv[h][:],
                        ones1,
                        bv_row[:, ts(h, 512)],
                        start=True,
                        stop=False,
                    )
                for k in range(KC):
                    for h in range(2):
                        nc.tensor.matmul(
                            psum_vv[h][:],
                            vft[:, k, :],
                            wv_t[k][:, ts(h, 512)],
                            start=False,
                            stop=(k == KC - 1),
                        )

                # ---- transpose vv -> vv^T chunks, then mm2 ------------------
                with (
                    tc.tile_pool(name="pt", bufs=1, space="PSUM") as pt,
                    tc.tile_pool(name="pr", bufs=1, space="PSUM") as pr,
                    tc.tile_pool(name="pb", bufs=1, space="PSUM") as pb,
                ):
                    psum_vvt = [
                        pt.tile([128, 4, B], bf16, name=f"pvt{h}", tag=f"pvt{h}")
                        for h in range(2)
                    ]
                    psum_row = pr.tile([B, CSH], f32, tag="pr")
                    psum_bc = pb.tile([128, B * CSH], f32, tag="pb")

                    # copy psum_vv -> SBUF bf16 (vector: half0, scalar: half1)
                    nc.vector.tensor_copy(vv_sb[0][:], psum_vv[0][:])
                    nc.scalar.copy(vv_sb[1][:], psum_vv[1][:])

                    for h in range(2):
                        for j in range(4):
                            nc.tensor.transpose(
                                psum_vvt[h][:, j, :],
                                vv_sb[h][0:B, ts(j, 128)],
                                ident,
                            )
                    nc.vector.tensor_copy(vvt_sb[0][:], psum_vvt[0][:])
                    nc.scalar.copy(vvt_sb[1][:], psum_vvt[1][:])

                    # mm2: prow = sum_k vvT_k^T @ Wp_k   [4, 128] f32
                    for k in range(KC):
                        nc.tensor.matmul(
                            psum_row[:],
                            vvt_sb[k // 4][:, k % 4, :],
                            wp_t[:, k, :],
                            start=(k == 0),
                            stop=(k == KC - 1),
                        )

                    # rhs5 rows 0-3 = rep4(prow) * sel   (bp already in row 4)
                    pra = psum_row[:]
                    prep = bass.AP(
                        pra.tensor,
                        pra.offset,
                        [list(pra.ap[0]), [0, B], list(pra.ap[1])],
                    )
                    nc.vector.tensor_mul(
                        rhs5_t[0:4, :].rearrange("p (q f) -> p q f", q=B),
                        prep,
                        sel.rearrange("p (q f) -> p q f", q=B),
                    )
                    # bcast: pbc[t, (b,c)] = ones5^T @ rhs5   (K=5, fp32)
                    nc.tensor.matmul(
                        psum_bc[:],
                        ones5,
                        rhs5_t[:],
                        start=True,
                        stop=True,
                    )
                    # copy psum_bc -> SBUF (vector: f-half0, scalar: f-half1)
                    nc.vector.tensor_copy(bc_t[:, 0:256], psum_bc[:, 0:256])
                    nc.scalar.copy(bc_t[:, 256:512], psum_bc[:, 256:512])

                    # out DMAs: replicated source over t-chunks; 2KB descs.
                    # sync writes q 0-3, scalar writes q 4-7.
                    out_v = out.rearrange("(q p) b c -> p q (b c)", p=128)
                    bca = bc_t[:]
                    rep = bass.AP(
                        bca.tensor,
                        bca.offset,
                        [list(bca.ap[0]), [0, KC // 2], list(bca.ap[1])],
                    )
                    nc.sync.dma_start(out_v[:, 0 : KC // 2, :], rep)
                    nc.scalar.dma_start(out_v[:, KC // 2 : KC, :], rep)

    nc.compile()
    return nc


def _get_built():
    global _BUILT
    if _BUILT is None:
        _BUILT = build_nc()
    return _BUILT


def make_in_maps(inputs):
    import ml_dtypes

    bf16 = ml_dtypes.bfloat16

    vf = np.asarray(inputs["visual_features"], np.float32)
    wv = np.asarray(inputs["Wv"], np.float32)
    wp = np.asarray(inputs["Wp"], np.float32)
    bv = np.asarray(inputs["bv"], np.float32)
    bp = np.asarray(inputs["bp"], np.float32)

    wv_bf = wv.astype(bf16)
    wv_chunks = [
        np.ascontiguousarray(wv_bf[k * 128 : (k + 1) * 128, :]) for k in range(KC)
    ]

    # vfti pack: vfT chunks + eye(4) + bv row + ones
    vfti = np.zeros((128, 1064), bf16)
    # vfti[p, k*4 + b] = vf[b, k*128 + p]
    vfti[:, 0:32] = (
        vf.T.reshape(KC, 128, B).transpose(1, 0, 2).reshape(128, KC * B)
    ).astype(bf16)
    vfti[0:4, 32:36] = np.eye(4, dtype=np.float32).astype(bf16)
    vfti[0:1, 36:1060] = bv[None, :].astype(bf16)
    vfti[0:1, 1060:1064] = np.ones((1, 4), np.float32).astype(bf16)

    # consts5: rows 0-3 cols 0:512 = sel, cols 512:640 = ones
    consts5 = np.zeros((5, 640), np.float32)
    for b in range(B):
        consts5[b, b * CSH : (b + 1) * CSH] = 1.0
    consts5[:, 512:640] = 1.0

    maps = []
    for i in range(N_CORES):
        ci = slice(i * CSH, (i + 1) * CSH)
        # wp_p[p, k*CSH + c] = Wp[k*128 + p, ci_c]
        wp_p = np.ascontiguousarray(
            wp[:, ci].reshape(KC, 128, CSH).transpose(1, 0, 2).reshape(128, KC * CSH)
        ).astype(bf16)
        bp_row = np.ascontiguousarray(np.tile(bp[ci], B)[None, :]).astype(np.float32)
        m = {
            "vfti": vfti,
            "wp_p": wp_p,
            "consts5": consts5,
            "bp_row": bp_row,
        }
        for k in range(KC):
            m[f"wv{k}"] = wv_chunks[k]
        maps.append(m)
    return maps


def run(inputs, trace=False, **kw):
    from concourse.bass_utils import run_bass_kernel_spmd

    nc = _get_built()
    res = run_bass_kernel_spmd(
        nc,
        make_in_maps(inputs),
        core_ids=list(range(N_CORES)),
        trace=trace,
        **kw,
    )
    full = np.empty((B, T, C), np.float32)
    for i, r in enumerate(res.results):
        full[:, :, i * CSH : (i + 1) * CSH] = r["out"].transpose(1, 0, 2)
    return full, res


def kernel(**inputs) -> np.ndarray:
    full, _ = run(inputs, trace=False)
    return full


# revision 5
# speedup vs baseline: 1.4005x; 1.0610x over previous
"""Trainium2 Bass kernel for nn_CrossAttention_47502338294587.

Math: the reference cross-attention has a single KV position broadcast over
all T query positions.  Softmax over a row of identical logits is uniform,
so attention output == v for every query, and the whole module collapses to

    out[b, t, :] = (visual_features[b] @ Wv + bv) @ Wp + bp      (for all t)

independent of x / Wq / Wk.  The device computes the two projections and
broadcasts the per-batch row over the T axis; the host only does input
layout prep (incl. bf16 weight packing) and shard re-assembly.

Sharding: tensor-parallel over the output channel dim C - core i computes
and writes out[:, :, i*128:(i+1)*128] (full Wv, column shard of Wp / bp).

v3: weights in bf16 (half the DMA bytes, single-pass matmuls), wv DMAs
issued first on both HWDGE queues (DMA issue instrs cost ~0.6-0.9us each),
PE kept warm through the DMA phase with K=1 dummy matmuls (HAM clock gate
otherwise halves the PE clock), mm1 chunks pipelined behind the DMA stream,
bv folded into the mm1 PSUM accumulation via a trailing K=1 matmul, bp
folded into the broadcast matmul via a 5th contraction row, broadcast
matmul in bf16, output DMA with 2KB descriptors split by t-chunk halves
across both queues.

Per-core structure:
  mm1:   psum_vv[h] = sum_k vfT_k^T @ Wv_k[:, h] + bv (K=1 matmul)  (bf16)
  tr:    vv -> vv^T chunks via PE transpose (bf16)
  mm2:   prow = sum_k vvT_k^T @ Wp_k          [4, 128] f32 psum
  bcast: rhs5[0:4] = rep4(prow) * sel (DVE), rhs5[4] = bp row (host const)
         pbc[t, (b,c)] = ones5^T @ rhs5  (one bf16 matmul, K=5)
         two replicated-source DMAs (t-chunks 0-3 / 4-7) write the shard
"""

import os
import sys

import numpy as np

for _p in ("/opt/trn_rl_repo",):
    if _p not in sys.path and os.path.isdir(_p):
        sys.path.insert(0, _p)

B, T, C = 4, 1024, 1024
N_CORES = 8
CSH = C // N_CORES  # 128, C-shard per core
KC = C // 128  # 8 contraction chunks
N_WARM = 10  # PE warmup dummy matmuls (HAM clock gate)

_BUILT = None


def build_nc():
    """Build + compile the Bass program (one NeuronCore's SPMD body)."""
    import concourse.bass as bass
    import concourse.mybir as mybir
    import concourse.tile as tile
    from concourse import bacc
    from concourse.bass import ts

    f32 = mybir.dt.float32
    bf16 = mybir.dt.bfloat16
    nc = bacc.Bacc("TRN2", target_bir_lowering=False, debug=False)

    # ---- DRAM inputs (host pre-packed layouts) --------------------------
    # wv_k[p, n] = bf16(Wv[k*128 + p, n])
    wv_d = [
        nc.dram_tensor(f"wv{k}", [128, C], bf16, kind="ExternalInput")
        for k in range(KC)
    ]
    # vfti[p, 0:32] = vfT chunks: [p, k*4 + b] = vf[b, k*128 + p]
    # vfti[0:4, 32:36] = eye(4); vfti[0:1, 36:40] = ones (K=1 bias lhsT)
    vfti_d = nc.dram_tensor("vfti", [128, 40], bf16, kind="ExternalInput")
    # bv row (rhs of the K=1 bias matmul)
    bv_d = nc.dram_tensor("bv_row", [1, C], bf16, kind="ExternalInput")
    # wp_p[p, k*CSH + c] = bf16(Wp[k*128 + p, ci_c])
    wp_d = nc.dram_tensor("wp_p", [128, KC * CSH], bf16, kind="ExternalInput")
    # consts5 rows 0-3 cols 0:512 = sel ((k==b) block mask), cols 512:640 ones
    consts_d = nc.dram_tensor("consts5", [5, 640], bf16, kind="ExternalInput")
    # bp_row[(b,c)] = bp[ci_c]  (tiled 4x) -> row 4 of rhs5
    bprow_d = nc.dram_tensor("bp_row", [1, B * CSH], bf16, kind="ExternalInput")
    # out[t, b, c_local]; host re-assembles full[b, t, ci] = out[t, b, :]
    out = nc.dram_tensor("out", [T, B, CSH], f32, kind="ExternalOutput")

    with tile.TileContext(nc) as tc:
        with (
            tc.tile_pool(name="sb", bufs=1) as sb,
            tc.tile_pool(name="pv", bufs=1, space="PSUM") as pv,
            tc.tile_pool(name="pt", bufs=1, space="PSUM") as pt,
            tc.tile_pool(name="pr", bufs=1, space="PSUM") as pr,
            tc.tile_pool(name="pb", bufs=1, space="PSUM") as pb,
        ):
            # ---- SBUF tiles -------------------------------------------------
            wv_t = [
                sb.tile([128, C], bf16, name=f"wv{k}", tag=f"wv{k}")
                for k in range(KC)
            ]
            vfti_t = sb.tile([128, 40], bf16, tag="vfti")
            bv_t = sb.tile([1, C], bf16, tag="bv_row")
            wp_t = sb.tile([128, KC, CSH], bf16, tag="wp_t")
            consts_t = sb.tile([5, 640], bf16, tag="consts5")
            vv_sb = [
                sb.tile([B, 512], bf16, name=f"vv{h}", tag=f"vv{h}")
                for h in range(2)
            ]
            vvt_sb = [
                sb.tile([128, 4, B], bf16, name=f"vvt{h}", tag=f"vvt{h}")
                for h in range(2)
            ]
            rhs5_t = sb.tile([5, B * CSH], bf16, tag="rhs5")
            bc_t = sb.tile([128, B * CSH], f32, tag="bc")
            warm_t = sb.tile([1, 640], bf16, tag="warm")

            vft = vfti_t[:, 0:32].rearrange("p (k b) -> p k b", b=B)
            ident = vfti_t[0:4, 32:36]
            ones1 = vfti_t[0:1, 36:40]
            sel = consts_t[0:4, 0:512]
            ones5 = consts_t[0:5, 512:640]

            # ---- PSUM tiles -------------------------------------------------
            psum_vv = [
                pv.tile([B, 512], f32, name=f"pvv{h}", tag=f"pvv{h}")
                for h in range(2)
            ]
            psum_vvt = [
                pt.tile([128, 4, B], bf16, name=f"pvt{h}", tag=f"pvt{h}")
                for h in range(2)
            ]
            psum_row = pr.tile([B, CSH], f32, tag="pr")
            psum_bc = pb.tile([128, B * CSH], f32, tag="pb")

            # ---- DMA in: two HWDGE queues, wv first -------------------------
            # sync queue: wv 0/2/4/6, consts5, bp_row (+ out q0-3 at the end)
            # scalar queue: vfti, wv 1/3/5/7, bv, wp (+ out q4-7 at the end)
            nc.scalar.dma_start(vfti_t[:], vfti_d[:, :])
            for k in range(KC):
                eng = nc.sync if k % 2 == 0 else nc.scalar
                eng.dma_start(wv_t[k][:], wv_d[k][:, :])
            nc.sync.dma_start(consts_t[:], consts_d[:, :])
            nc.sync.dma_start(rhs5_t[4:5, :], bprow_d[:, :])
            nc.scalar.dma_start(bv_t[:], bv_d[:, :])
            nc.scalar.dma_start(
                wp_t[:], wp_d.rearrange("p (k c) -> p k c", c=CSH)
            )

            # ---- PE warmup: K=1 dummies into psum_bc (overwritten later) ----
            nc.gpsimd.memset(warm_t[:], 1.0)
            for w in range(N_WARM):
                nc.tensor.matmul(
                    psum_bc[:, 0:512],
                    warm_t[0:1, 0:128],
                    warm_t[0:1, 128:640],
                    start=True,
                    stop=True,
                )

            # ---- mm1: psum_vv[h] = sum_k vfT_k^T @ Wv_k[:, h] + bv ----------
            for k in range(KC):
                for h in range(2):
                    nc.tensor.matmul(
                        psum_vv[h][:],
                        vft[:, k, :],
                        wv_t[k][:, ts(h, 512)],
                        start=(k == 0),
                        stop=False,
                    )
            # trailing K=1 bias row
            for h in range(2):
                nc.tensor.matmul(
                    psum_vv[h][:],
                    ones1,
                    bv_t[0:1, ts(h, 512)],
                    start=False,
                    stop=True,
                )

            # ---- transpose vv -> vv^T chunks, then mm2 ----------------------
            # copy psum_vv -> SBUF bf16 (vector: half0, scalar: half1)
            nc.vector.tensor_copy(vv_sb[0][:], psum_vv[0][:])
            nc.scalar.copy(vv_sb[1][:], psum_vv[1][:])

            for h in range(2):
                for j in range(4):
                    nc.tensor.transpose(
                        psum_vvt[h][:, j, :],
                        vv_sb[h][0:B, ts(j, 128)],
                        ident,
                    )
            nc.vector.tensor_copy(vvt_sb[0][:], psum_vvt[0][:])
            nc.scalar.copy(vvt_sb[1][:], psum_vvt[1][:])

            # mm2: prow = sum_k vvT_k^T @ Wp_k   [4, 128] f32
            for k in range(KC):
                nc.tensor.matmul(
                    psum_row[:],
                    vvt_sb[k // 4][:, k % 4, :],
                    wp_t[:, k, :],
                    start=(k == 0),
                    stop=(k == KC - 1),
                )

            # rhs5 rows 0-3 = rep4(prow) * sel   (bp already in row 4)
            pra = psum_row[:]
            prep = bass.AP(
                pra.tensor,
                pra.offset,
                [list(pra.ap[0]), [0, B], list(pra.ap[1])],
            )
            nc.vector.tensor_mul(
                rhs5_t[0:4, :].rearrange("p (q f) -> p q f", q=B),
                prep,
                sel.rearrange("p (q f) -> p q f", q=B),
            )
            # bcast: pbc[t, (b,c)] = ones5^T @ rhs5   (K=5, bf16)
            nc.tensor.matmul(
                psum_bc[:],
                ones5,
                rhs5_t[:],
                start=True,
                stop=True,
            )
            # copy psum_bc -> SBUF (vector: f-half0, scalar: f-half1)
            nc.vector.tensor_copy(bc_t[:, 0:256], psum_bc[:, 0:256])
            nc.scalar.copy(bc_t[:, 256:512], psum_bc[:, 256:512])

            # out DMAs: replicated source over t-chunks; 2KB descs.
            # sync writes q 0-3, scalar writes q 4-7.
            out_v = out.rearrange("(q p) b c -> p q (b c)", p=128)
            bca = bc_t[:]
            rep = bass.AP(
                bca.tensor,
                bca.offset,
                [list(bca.ap[0]), [0, KC // 2], list(bca.ap[1])],
            )
            nc.sync.dma_start(out_v[:, 0 : KC // 2, :], rep)
            nc.scalar.dma_start(out_v[:, KC // 2 : KC, :], rep)

    nc.compile()
    return nc


def _get_built():
    global _BUILT
    if _BUILT is None:
        _BUILT = build_nc()
    return _BUILT


def make_in_maps(inputs):
    import ml_dtypes

    bf16 = ml_dtypes.bfloat16

    vf = np.asarray(inputs["visual_features"], np.float32)
    wv = np.asarray(inputs["Wv"], np.float32)
    wp = np.asarray(inputs["Wp"], np.float32)
    bv = np.asarray(inputs["bv"], np.float32)
    bp = np.asarray(inputs["bp"], np.float32)

    wv_bf = wv.astype(bf16)
    wv_chunks = [
        np.ascontiguousarray(wv_bf[k * 128 : (k + 1) * 128, :]) for k in range(KC)
    ]

    # vfti pack: vfT chunks + eye(4) + ones row
    vfti = np.zeros((128, 40), np.float32)
    vfti[:, 0:32] = vf.T.reshape(KC, 128, B).transpose(1, 0, 2).reshape(128, KC * B)
    vfti[0:4, 32:36] = np.eye(4, dtype=np.float32)
    vfti[0:1, 36:40] = 1.0
    vfti = vfti.astype(bf16)

    bv_row = np.ascontiguousarray(bv[None, :]).astype(bf16)

    # consts5: rows 0-3 cols 0:512 = sel, cols 512:640 = ones
    consts5 = np.zeros((5, 640), np.float32)
    for b in range(B):
        consts5[b, b * CSH : (b + 1) * CSH] = 1.0
    consts5[:, 512:640] = 1.0
    consts5 = consts5.astype(bf16)

    maps = []
    for i in range(N_CORES):
        ci = slice(i * CSH, (i + 1) * CSH)
        # wp_p[p, k*CSH + c] = Wp[k*128 + p, ci_c]
        wp_p = np.ascontiguousarray(
            wp[:, ci].reshape(KC, 128, CSH).transpose(1, 0, 2).reshape(128, KC * CSH)
        ).astype(bf16)
        bp_row = np.ascontiguousarray(np.tile(bp[ci], B)[None, :]).astype(bf16)
        m = {
            "vfti": vfti,
            "bv_row": bv_row,
            "wp_p": wp_p,
            "consts5": consts5,
            "bp_row": bp_row,
        }
        for k in range(KC):
            m[f"wv{k}"] = wv_chunks[k]
        maps.append(m)
    return maps


def run(inputs, trace=False, **kw):
    from concourse.bass_utils import run_bass_kernel_spmd

    nc = _get_built()
    res = run_bass_kernel_spmd(
        nc,
        make_in_maps(inputs),
        core_ids=list(range(N_CORES)),
        trace=trace,
        **kw,
    )
    full = np.empty((B, T, C), np.float32)
    for i, r in enumerate(res.results):
        full[:, :, i * CSH : (i + 1) * CSH] = r["out"].transpose(1, 0, 2)
    return full, res


def kernel(**inputs) -> np.ndarray:
    full, _ = run(inputs, trace=False)
    return full


# revision 6
# speedup vs baseline: 1.4241x; 1.0168x over previous
"""Trainium2 Bass kernel for nn_CrossAttention_47502338294587.

Math: the reference cross-attention has a single KV position broadcast over
all T query positions.  Softmax over a row of identical logits is uniform,
so attention output == v for every query, and the whole module collapses to

    out[b, t, :] = (visual_features[b] @ Wv + bv) @ Wp + bp      (for all t)

independent of x / Wq / Wk.  The device computes the two projections and
broadcasts the per-batch row over the T axis; the host only does input
layout prep (incl. bf16 weight packing) and shard re-assembly.

Sharding: tensor-parallel over the output channel dim C - core i computes
and writes out[:, :, i*128:(i+1)*128] (full Wv, column shard of Wp / bp).

v4: the two HWDGE queues carry only the wv chunk stream (+ tiny vfti);
wp and a packed consts tensor (sel / bp row / ones / bv) ride the idle
gpsimd SWDGE queue.  PE kept warm with early K=1 dummy matmuls (HAM clock
gate).  bv enters the mm1 PSUM group as the leading K=1 start=True matmul
so the group completes right at the last wv chunk.  bp sits pre-placed in
row 4 of the sel region, which the DVE rhs5 multiply updates in place, so
the broadcast matmul (bf16, K=5) reads sel/bp directly.  All PSUM->SBUF
copies on vector (no ACT tables on the scalar engine).  Output: four
replicated-source DMAs (t-chunk pairs) with 2KB descriptors.

Per-core structure:
  mm1:   psum_vv[h] = bv (K=1, start) + sum_k vfT_k^T @ Wv_k[:, h]  (bf16)
  tr:    vv -> vv^T chunks via PE transpose (bf16)
  mm2:   prow = sum_k vvT_k^T @ Wp_k          [4, 128] f32 psum
  bcast: sel[0:4] *= rep4(prow) in place (DVE), sel row 4 = bp (host)
         pbc[t, (b,c)] = ones5^T @ sel5  (one bf16 matmul, K=5)
         four replicated-source DMAs (t-chunk pairs) write the shard
"""

import os
import sys

import numpy as np

for _p in ("/opt/trn_rl_repo",):
    if _p not in sys.path and os.path.isdir(_p):
        sys.path.insert(0, _p)

B, T, C = 4, 1024, 1024
N_CORES = 8
CSH = C // N_CORES  # 128, C-shard per core
KC = C // 128  # 8 contraction chunks
N_WARM = 10  # PE warmup dummy matmuls (HAM clock gate)

_BUILT = None


def build_nc():
    """Build + compile the Bass program (one NeuronCore's SPMD body)."""
    import concourse.bass as bass
    import concourse.mybir as mybir
    import concourse.tile as tile
    from concourse import bacc
    from concourse.bass import ts

    f32 = mybir.dt.float32
    bf16 = mybir.dt.bfloat16
    nc = bacc.Bacc("TRN2", target_bir_lowering=False, debug=False)

    # ---- DRAM inputs (host pre-packed layouts) --------------------------
    # wv_k[p, n] = bf16(Wv[k*128 + p, n])
    wv_d = [
        nc.dram_tensor(f"wv{k}", [128, C], bf16, kind="ExternalInput")
        for k in range(KC)
    ]
    # vfti[p, 0:32] = vfT chunks: [p, k*4 + b] = vf[b, k*128 + p]
    # vfti[0:4, 32:36] = eye(4); vfti[0:1, 36:40] = ones (K=1 bias lhsT)
    vfti_d = nc.dram_tensor("vfti", [128, 40], bf16, kind="ExternalInput")
    # wp_p[p, k*CSH + c] = bf16(Wp[k*128 + p, ci_c])
    wp_d = nc.dram_tensor("wp_p", [128, KC * CSH], bf16, kind="ExternalInput")
    # consts pack [5, 1664]:
    #   [0:4, 0:512]   sel ((k==b) block mask; becomes rhs5 rows 0-3 in place)
    #   [4:5, 0:512]   bp row (tiled 4x) = rhs5 row 4
    #   [0:5, 512:640] ones5
    #   [0:1, 640:1664] bv row
    consts_d = nc.dram_tensor("consts5", [5, 1664], bf16, kind="ExternalInput")
    # out[t, b, c_local]; host re-assembles full[b, t, ci] = out[t, b, :]
    out = nc.dram_tensor("out", [T, B, CSH], f32, kind="ExternalOutput")

    with tile.TileContext(nc) as tc:
        with (
            tc.tile_pool(name="sb", bufs=1) as sb,
            tc.tile_pool(name="pv", bufs=1, space="PSUM") as pv,
            tc.tile_pool(name="pt", bufs=1, space="PSUM") as pt,
            tc.tile_pool(name="pr", bufs=1, space="PSUM") as pr,
            tc.tile_pool(name="pb", bufs=1, space="PSUM") as pb,
        ):
            # ---- SBUF tiles -------------------------------------------------
            wv_t = [
                sb.tile([128, C], bf16, name=f"wv{k}", tag=f"wv{k}")
                for k in range(KC)
            ]
            vfti_t = sb.tile([128, 40], bf16, tag="vfti")
            wp_t = sb.tile([128, KC, CSH], bf16, tag="wp_t")
            consts_t = sb.tile([5, 1664], bf16, tag="consts5")
            vv_sb = [
                sb.tile([B, 512], bf16, name=f"vv{h}", tag=f"vv{h}")
                for h in range(2)
            ]
            vvt_sb = [
                sb.tile([128, 4, B], bf16, name=f"vvt{h}", tag=f"vvt{h}")
                for h in range(2)
            ]
            bc_t = sb.tile([128, B * CSH], f32, tag="bc")
            warm_t = sb.tile([128, 640], bf16, tag="warm")

            vft = vfti_t[:, 0:32].rearrange("p (k b) -> p k b", b=B)
            ident = vfti_t[0:4, 32:36]
            ones1 = vfti_t[0:1, 36:40]
            sel5 = consts_t[0:5, 0:512]
            sel4 = consts_t[0:4, 0:512]
            ones5 = consts_t[0:5, 512:640]
            bv_row = consts_t[0:1, 640:1664]

            # ---- PSUM tiles -------------------------------------------------
            psum_vv = [
                pv.tile([B, 512], f32, name=f"pvv{h}", tag=f"pvv{h}")
                for h in range(2)
            ]
            psum_vvt = [
                pt.tile([128, 4, B], bf16, name=f"pvt{h}", tag=f"pvt{h}")
                for h in range(2)
            ]
            psum_row = pr.tile([B, CSH], f32, tag="pr")
            psum_bc = pb.tile([128, B * CSH], f32, tag="pb")

            # ---- DMA in -----------------------------------------------------
            # sync HWDGE queue: wv 0/2/4/6 (+ out q01, q23 at the end)
            # scalar HWDGE queue: vfti, wv 1/3/5/7 (+ out q45, q67 at the end)
            # gpsimd SWDGE queue: consts pack, wp
            nc.scalar.dma_start(vfti_t[:], vfti_d[:, :])
            for k in range(KC):
                eng = nc.sync if k % 2 == 0 else nc.scalar
                eng.dma_start(wv_t[k][:], wv_d[k][:, :])
            nc.gpsimd.dma_start(consts_t[:], consts_d[:, :])
            nc.gpsimd.dma_start(
                wp_t[:], wp_d.rearrange("p (k c) -> p k c", c=CSH)
            )

            # ---- PE warmup: K=1 dummies into psum_bc (overwritten later) ----
            nc.gpsimd.memset(warm_t[:], 1.0)
            for w in range(N_WARM):
                nc.tensor.matmul(
                    psum_bc[:, 0:512],
                    warm_t[0:1, 0:128],
                    warm_t[0:1, 128:640],
                    start=True,
                    stop=True,
                )

            # ---- mm1: psum_vv[h] = bv + sum_k vfT_k^T @ Wv_k[:, h] ----------
            # leading K=1 bias row opens the accumulation group
            for h in range(2):
                nc.tensor.matmul(
                    psum_vv[h][:],
                    ones1,
                    bv_row[:, ts(h, 512)],
                    start=True,
                    stop=False,
                )
            for k in range(KC):
                for h in range(2):
                    nc.tensor.matmul(
                        psum_vv[h][:],
                        vft[:, k, :],
                        wv_t[k][:, ts(h, 512)],
                        start=False,
                        stop=(k == KC - 1),
                    )

            # ---- transpose vv -> vv^T chunks, then mm2 ----------------------
            nc.vector.tensor_copy(vv_sb[0][:], psum_vv[0][:])
            nc.vector.tensor_copy(vv_sb[1][:], psum_vv[1][:])

            for h in range(2):
                for j in range(4):
                    nc.tensor.transpose(
                        psum_vvt[h][:, j, :],
                        vv_sb[h][0:B, ts(j, 128)],
                        ident,
                    )
            nc.vector.tensor_copy(vvt_sb[0][:], psum_vvt[0][:])
            nc.vector.tensor_copy(vvt_sb[1][:], psum_vvt[1][:])

            # mm2: prow = sum_k vvT_k^T @ Wp_k   [4, 128] f32
            for k in range(KC):
                nc.tensor.matmul(
                    psum_row[:],
                    vvt_sb[k // 4][:, k % 4, :],
                    wp_t[:, k, :],
                    start=(k == 0),
                    stop=(k == KC - 1),
                )

            # sel rows 0-3 *= rep4(prow) in place (row 4 = bp, host-placed)
            pra = psum_row[:]
            prep = bass.AP(
                pra.tensor,
                pra.offset,
                [list(pra.ap[0]), [0, B], list(pra.ap[1])],
            )
            nc.vector.tensor_mul(
                sel4.rearrange("p (q f) -> p q f", q=B),
                prep,
                sel4.rearrange("p (q f) -> p q f", q=B),
            )
            # bcast: pbc[t, (b,c)] = ones5^T @ sel5   (K=5, bf16)
            nc.tensor.matmul(
                psum_bc[:],
                ones5,
                sel5,
                start=True,
                stop=True,
            )
            nc.vector.tensor_copy(bc_t[:], psum_bc[:])

            # out DMAs: replicated source over t-chunk pairs; 2KB descs.
            out_v = out.rearrange("(q p) b c -> p q (b c)", p=128)
            bca = bc_t[:]
            rep = bass.AP(
                bca.tensor,
                bca.offset,
                [list(bca.ap[0]), [0, 2], list(bca.ap[1])],
            )
            for i, eng in ((0, nc.sync), (2, nc.scalar), (1, nc.sync), (3, nc.scalar)):
                eng.dma_start(out_v[:, 2 * i : 2 * i + 2, :], rep)

    nc.compile()
    return nc


def _get_built():
    global _BUILT
    if _BUILT is None:
        _BUILT = build_nc()
    return _BUILT


def make_in_maps(inputs):
    import ml_dtypes

    bf16 = ml_dtypes.bfloat16

    vf = np.asarray(inputs["visual_features"], np.float32)
    wv = np.asarray(inputs["Wv"], np.float32)
    wp = np.asarray(inputs["Wp"], np.float32)
    bv = np.asarray(inputs["bv"], np.float32)
    bp = np.asarray(inputs["bp"], np.float32)

    wv_bf = wv.astype(bf16)
    wv_chunks = [
        np.ascontiguousarray(wv_bf[k * 128 : (k + 1) * 128, :]) for k in range(KC)
    ]

    # vfti pack: vfT chunks + eye(4) + ones row
    vfti = np.zeros((128, 40), np.float32)
    vfti[:, 0:32] = vf.T.reshape(KC, 128, B).transpose(1, 0, 2).reshape(128, KC * B)
    vfti[0:4, 32:36] = np.eye(4, dtype=np.float32)
    vfti[0:1, 36:40] = 1.0
    vfti = vfti.astype(bf16)

    # consts pack: sel + bp row + ones5 + bv row (bp per-core, rest shared)
    consts_base = np.zeros((5, 1664), np.float32)
    for b in range(B):
        consts_base[b, b * CSH : (b + 1) * CSH] = 1.0
    consts_base[:, 512:640] = 1.0
    consts_base[0, 640:1664] = bv

    maps = []
    for i in range(N_CORES):
        ci = slice(i * CSH, (i + 1) * CSH)
        # wp_p[p, k*CSH + c] = Wp[k*128 + p, ci_c]
        wp_p = np.ascontiguousarray(
            wp[:, ci].reshape(KC, 128, CSH).transpose(1, 0, 2).reshape(128, KC * CSH)
        ).astype(bf16)
        consts5 = consts_base.copy()
        consts5[4, 0:512] = np.tile(bp[ci], B)
        m = {
            "vfti": vfti,
            "wp_p": wp_p,
            "consts5": consts5.astype(bf16),
        }
        for k in range(KC):
            m[f"wv{k}"] = wv_chunks[k]
        maps.append(m)
    return maps


def run(inputs, trace=False, **kw):
    from concourse.bass_utils import run_bass_kernel_spmd

    nc = _get_built()
    res = run_bass_kernel_spmd(
        nc,
        make_in_maps(inputs),
        core_ids=list(range(N_CORES)),
        trace=trace,
        **kw,
    )
    full = np.empty((B, T, C), np.float32)
    for i, r in enumerate(res.results):
        full[:, :, i * CSH : (i + 1) * CSH] = r["out"].transpose(1, 0, 2)
    return full, res


def kernel(**inputs) -> np.ndarray:
    full, _ = run(inputs, trace=False)
    return full
